# revision 11
# baseline (speedup 1.0000x reference)
"""Trainium2 Bass kernel for nn_AttentionLayer (B=16, T=2048, D=256), 8 cores.

Math (per batch b):
    h  = input[:, :256] + input[:, 256:512]            # [T, D]
    aw = relu(h @ W.T + b)                             # [T, D]
    m  = tanh(h)
    S  = m @ aw.T                                      # [T, T]
    P  = softmax(S, axis=-1)
    out = h.T + h.T @ P                                # [D, T]

Sharding: data-parallel over batch. 16 batches -> 2 per NeuronCore.

Kernel structure per batch (all bf16 on the TensorEngine, f32 PSUM accum):
    S1 : DMA input row-blocks [128, 512], h_td = half0 + half1 (bf16).
    S1b: transpose h_td via identity-matmul -> hT; mT = tanh(hT) (ScalarE).
    S2 : awT = relu(W.T-matmul + bias) (bias+relu fused on VectorE).
    S3 : per t-block: S = mT.T @ awT (PSUM), E = exp(S - 45) on ScalarE with
         fused row-sum accumulation; g = h_td / rowsum (DVE).  The constant
         -45 shift is mathematically exact for softmax (cancels in the
         normalization) and keeps exp() in range without a row-max pass.
    S4 : out[dh, sc] = sum_t g[t, d] * E[t, s] accumulated in PSUM over the
         16 t-blocks; residual h.T added during the PSUM->SBUF evacuation.

Software pipeline: S4 of batch b-1 is emitted interleaved into S3 of batch
b, so TensorE's S4 accumulation absorbs ScalarE's exp() surplus (ScalarE is
slightly slower per t-block than TensorE during S3).  PSUM: 'ps' pool
(2 x [128,1024] = 4 banks) cycles S1b/S2/S3 tiles; 'pso' pool (4 banks)
holds S4 accumulators.
"""

import numpy as np

import concourse.bass as bass
import concourse.mybir as mybir
import concourse.tile as tile
from concourse import bacc
from concourse.bass_utils import run_bass_kernel_spmd
from concourse.masks import make_identity

N_CORES = 8
EXP_SHIFT = -45.0  # exact for softmax; bounds exp() inputs


def build_kernel(nc, b_loc: int, t: int, d: int):
    """Emit the Tile program. t = seq len, d = feature dim (256)."""
    f32 = mybir.dt.float32
    bf16 = mybir.dt.bfloat16
    P = 128
    MMN = min(512, t)     # moving-operand width per matmul
    SC = min(1024, t)     # psum tile / ACT-op width
    ntb = t // P          # t-blocks per batch
    d_halves = d // P     # 2
    nsc = t // SC         # s-chunks per row

    inp = nc.dram_tensor("input_feature", [b_loc, t, 2 * d], f32,
                         kind="ExternalInput").ap()
    W = nc.dram_tensor("W", [d, d], f32, kind="ExternalInput").ap()
    bias = nc.dram_tensor("b", [d], f32, kind="ExternalInput").ap()
    out = nc.dram_tensor("out", [b_loc, d, t], f32,
                         kind="ExternalOutput").ap()

    with tile.TileContext(nc) as tc:
        with (
            tc.tile_pool(name="const", bufs=1) as const,
            tc.tile_pool(name="ps", bufs=2, space="PSUM") as ps,
            tc.tile_pool(name="pso", bufs=2, space="PSUM") as pso,
            tc.tile_pool(name="inp_p", bufs=4) as inp_p,
            tc.tile_pool(name="h_p", bufs=2 * ntb) as h_p,
            tc.tile_pool(name="g_p", bufs=2 * ntb) as g_p,
            tc.tile_pool(name="hT_p", bufs=2 * d_halves) as hT_p,
            tc.tile_pool(name="mT_p", bufs=2 * d_halves) as mT_p,
            tc.tile_pool(name="awT_p", bufs=2 * d_halves) as awT_p,
            tc.tile_pool(name="E_p", bufs=min(2 * ntb, ntb + 8)) as E_p,
            tc.tile_pool(name="z_p", bufs=8) as z_p,
            tc.tile_pool(name="out_p", bufs=3) as out_p,
        ):
            # ---- setup: identity, W^T (bf16), bias ----
            ident = const.tile([P, P], bf16)
            make_identity(nc, ident[:])

            b_sb = const.tile([P, d_halves], f32)
            nc.sync.dma_start(out=b_sb[:], in_=bias.rearrange("(h p) -> p h", p=P))

            shift = const.tile([P, 1], f32)
            nc.vector.memset(shift[:], EXP_SHIFT)

            w_stage = []
            for k in range(d_halves):
                wf = const.tile([P, d], f32, tag=f"w_f32_{k}")
                nc.sync.dma_start(out=wf[:], in_=W[k * P:(k + 1) * P, :])
                wb = const.tile([P, d], bf16, tag=f"w_bf16_{k}")
                nc.vector.tensor_copy(wb[:], wf[:])
                w_stage.append(wb)

            WT = []
            for dh in range(d_halves):
                ps_w = ps.tile([P, SC], f32, tag="ps", name=f"ps_w{dh}")
                for k in range(d_halves):
                    nc.tensor.matmul(ps_w[:, k * P:(k + 1) * P],
                                     w_stage[k][:, dh * P:(dh + 1) * P],
                                     ident[:], start=True, stop=True)
                wt = const.tile([P, d], bf16, tag=f"wt_{dh}")
                nc.vector.tensor_copy(wt[:], ps_w[:, 0:d])
                WT.append(wt)

            state = {}   # per-batch tiles needed by the deferred S4
            pending = [] # S4 regions of the previous batch awaiting emission

            def emit_s4_region(pb, dh, sc):
                st = state[pb]
                ps_o = pso.tile([P, SC], f32, tag="pso", name=f"pso{pb}_{dh}_{sc}")
                for n0 in range(sc * SC, (sc + 1) * SC, MMN):
                    for tb in range(ntb):
                        nc.tensor.matmul(
                            ps_o[:, n0 - sc * SC:n0 - sc * SC + MMN],
                            st["g"][tb][:, dh * P:(dh + 1) * P],
                            st["E"][tb][:, n0:n0 + MMN],
                            start=(tb == 0), stop=(tb == ntb - 1))
                ot = out_p.tile([P, SC], f32, tag="out", name=f"ot{pb}_{dh}_{sc}")
                nc.vector.tensor_add(ot[:], ps_o[:],
                                     st["hT"][dh][:, sc * SC:(sc + 1) * SC])
                nc.sync.dma_start(
                    out=out[pb, dh * P:(dh + 1) * P, sc * SC:(sc + 1) * SC],
                    in_=ot[:])

            for b in range(b_loc):
                # ---- S1: load input, h = h1 + h2 (bf16, t on partitions) ----
                h_td = []
                for tb in range(ntb):
                    it = inp_p.tile([P, 2 * d], f32, tag="in")
                    nc.sync.dma_start(out=it[:], in_=inp[b, tb * P:(tb + 1) * P, :])
                    ht = h_p.tile([P, d], bf16, tag="h")
                    nc.vector.tensor_add(ht[:], it[:, 0:d], it[:, d:2 * d])
                    h_td.append(ht)

                # ---- S1b: transpose -> hT (bf16), mT = tanh(hT) ----
                hT = [hT_p.tile([P, t], bf16, tag="hT", name=f"hT{i}")
                      for i in range(d_halves)]
                mT = [mT_p.tile([P, t], bf16, tag="mT", name=f"mT{i}")
                      for i in range(d_halves)]
                for dh in range(d_halves):
                    for q in range(nsc):
                        ps_t = ps.tile([P, SC], f32, tag="ps", name=f"ps_t{dh}_{q}")
                        for j in range(SC // P):
                            tb = q * (SC // P) + j
                            nc.tensor.matmul(ps_t[:, j * P:(j + 1) * P],
                                             h_td[tb][:, dh * P:(dh + 1) * P],
                                             ident[:], start=True, stop=True)
                        sl = slice(q * SC, (q + 1) * SC)
                        nc.scalar.activation(mT[dh][:, sl], ps_t[:],
                                             mybir.ActivationFunctionType.Tanh)
                        nc.vector.tensor_copy(hT[dh][:, sl], ps_t[:])

                # ---- S2: awT = relu(W.T @ h.T + b) ----
                awT = [awT_p.tile([P, t], bf16, tag="awT", name=f"awT{i}")
                       for i in range(d_halves)]
                for eh in range(d_halves):
                    for sc in range(nsc):
                        ps_aw = ps.tile([P, SC], f32, tag="ps", name=f"ps_aw{eh}_{sc}")
                        for n0 in range(0, SC, MMN):
                            for k in range(d_halves):
                                nc.tensor.matmul(
                                    ps_aw[:, n0:n0 + MMN],
                                    WT[k][:, eh * P:(eh + 1) * P],
                                    hT[k][:, sc * SC + n0:sc * SC + n0 + MMN],
                                    start=(k == 0), stop=(k == d_halves - 1))
                        nc.vector.tensor_scalar(
                            out=awT[eh][:, sc * SC:(sc + 1) * SC], in0=ps_aw[:],
                            scalar1=b_sb[:, eh:eh + 1], scalar2=0.0,
                            op0=mybir.AluOpType.add, op1=mybir.AluOpType.max)

                # ---- S3 (batch b) with S4 (batch b-1) interleaved ----
                E = [E_p.tile([P, t], bf16, tag="E", name=f"E{i}")
                     for i in range(ntb)]
                g = [g_p.tile([P, d], bf16, tag="g", name=f"g{i}")
                     for i in range(ntb)]
                state[b] = {"E": E, "g": g, "hT": hT, "h": h_td}
                ri = 0
                for tb in range(ntb):
                    tsl = slice(tb * P, (tb + 1) * P)
                    zp = z_p.tile([P, nsc], f32, tag="zp")
                    for sc in range(nsc):
                        ps_s = ps.tile([P, SC], f32, tag="ps", name=f"ps_s{tb}_{sc}")
                        for n0 in range(0, SC, MMN):
                            for k in range(d_halves):
                                nc.tensor.matmul(
                                    ps_s[:, n0:n0 + MMN], mT[k][:, tsl],
                                    awT[k][:, sc * SC + n0:sc * SC + n0 + MMN],
                                    start=(k == 0), stop=(k == d_halves - 1))
                        nc.scalar.activation(E[tb][:, sc * SC:(sc + 1) * SC],
                                             ps_s[:],
                                             mybir.ActivationFunctionType.Exp,
                                             bias=shift[:], scale=1.0,
                                             accum_out=zp[:, sc:sc + 1])
                    if nsc == 1:
                        zs = zp
                    else:
                        zs = z_p.tile([P, 1], f32, tag="zs")
                        nc.vector.tensor_add(zs[:], zp[:, 0:1], zp[:, 1:2])
                    rinv = z_p.tile([P, 1], f32, tag="rinv")
                    nc.vector.reciprocal(rinv[:], zs[:])
                    nc.vector.tensor_scalar_mul(g[tb][:], h_td[tb][:], rinv[:])

                    # interleave one deferred S4 region after every other tb
                    if pending and tb % 2 == 1:
                        emit_s4_region(*pending.pop(0))
                while pending:
                    emit_s4_region(*pending.pop(0))
                if b > 0:
                    del state[b - 1]
                pending = [(b, dh, sc)
                           for dh in range(d_halves) for sc in range(nsc)]

            while pending:
                emit_s4_region(*pending.pop(0))
    return nc


_COMPILED = {}


def _get_compiled(b_loc: int, t: int, d: int):
    key = (b_loc, t, d)
    if key not in _COMPILED:
        nc = bacc.Bacc("TRN2", target_bir_lowering=False, debug=False,
                       num_devices=N_CORES)
        build_kernel(nc, b_loc, t, d)
        nc.compile()
        _COMPILED[key] = nc
    return _COMPILED[key]


def kernel(input_feature: np.ndarray, W: np.ndarray, b: np.ndarray,
           trace: bool = False, **extra_kwargs):
    input_feature = np.ascontiguousarray(input_feature, dtype=np.float32)
    W = np.ascontiguousarray(W, dtype=np.float32)
    b = np.ascontiguousarray(b, dtype=np.float32)

    b_full, t, d2 = input_feature.shape
    b_loc = b_full // N_CORES
    nc = _get_compiled(b_loc, t, d2 // 2)

    in_maps = [
        {"input_feature": input_feature[i * b_loc:(i + 1) * b_loc], "W": W, "b": b}
        for i in range(N_CORES)
    ]
    res = run_bass_kernel_spmd(nc, in_maps, core_ids=list(range(N_CORES)),
                               trace=trace, **extra_kwargs)
    out = np.concatenate([r["out"] for r in res.results], axis=0)
    if trace:
        kernel.last_result = res
    return out
